# revision 1
# baseline (speedup 1.0000x reference)
"""2-layer GCN (PyG GCNConv x2 + ReLU) on 8 Trainium2 NeuronCores.

Strategy (graph/data parallel over destination nodes):
  - out = A_hat @ (X @ W) == (A_hat @ X) @ W  (aggregation commutes with the
    linear map), so layer 1 aggregates raw 128-dim x rows (512 B gathers) and
    layer 2 aggregates h2 = relu(out1) @ W2 rows (64-dim, 256 B gathers).
  - Each core owns a contiguous dst range of 12500 nodes (padded to
    12544 = 98 windows x 128). It processes exactly the edges whose dst lands
    in its range. Edge aggregation per 128-dst window accumulates in PSUM:
    for each 128-edge chunk, indirect-DMA gather the source rows
    (partition = edge), build the one-hot scatter matrix
    S[e, d] = norm_e * (dst_in_window_e == d) with one chained DVE
    tensor_scalar op over an iota tile, and matmul lhsT=M rhs=S accumulating
    [feat, dst] into PSUM (layer 1) / lhsT=S rhs=M2 into [dst, feat] (layer 2).
  - Between layers, one 8-rank AllGather shares each core's h2 shard
    ([12544, 64] f32, 3.2 MB) so layer-2 gathers can read any node.
  - Weights are tiny and replicated; biases enter as rank-1 matmuls into the
    same PSUM accumulation group.

Host-side preprocessing (numpy): degrees/normalization, partition edges by
dst core, sort by (window), pack into fixed-size 128-edge chunks (K chunks
per window, K = global max so the SPMD program is identical on all cores;
padding slots have norm=0 so they contribute nothing).
"""
import sys
import time

sys.path.insert(0, "/opt/trn_rl_repo")

import numpy as np
from ml_dtypes import bfloat16

import concourse.bass as bass
import concourse.mybir as mybir
from concourse.tile import TileContext
from concourse.tile_rust import add_dep_helper
from concourse import bass_utils

P = 128
N_NODES = 100000
NCORES = 8
D_CORE = N_NODES // NCORES          # 12500
WINDOWS = (D_CORE + P - 1) // P     # 98
D_PAD = WINDOWS * P                 # 12544
N_PAD = NCORES * D_PAD              # 100352
D_IN, HID, D_OUT = 128, 256, 64

# set by test.py to capture profiling info
TRACE = False
LAST_EXEC_NS = None
LAST_RESULTS = None

_F32 = mybir.dt.float32
_BF16 = mybir.dt.bfloat16
_I32 = mybir.dt.int32


def _split_multi_waits(nc):
    """walrus here refuses instructions with >1 sem wait on several ISA
    structs; split extras into standalone EventSemaphore instructions."""
    for f in nc.m.functions:
        for b in f.blocks:
            out = []
            for inst in b.instructions:
                si = inst.sync_info
                if si is not None and len(si.on_wait) > 1:
                    waits = list(si.on_wait)
                    for j, w in enumerate(waits[:-1]):
                        ev = mybir.InstEventSemaphore(
                            name=f"{inst.name}-wsplit{j}", ins=[], outs=[]
                        )
                        ev.engine = inst.engine
                        ev.sync_info = mybir.SyncInfo(on_wait=[w], on_update=[])
                        out.append(ev)
                    inst.sync_info = mybir.SyncInfo(
                        on_wait=[waits[-1]], on_update=list(si.on_update)
                    )
                out.append(inst)
            b.instructions = out


def _build_schedule(edge_index):
    """Pack edges (incl. self-loops) into per-core fixed-shape chunk arrays.

    Returns K and arrays of shape [NCORES, P, WINDOWS*K]:
      idx1: int32 gather indices into x  [N_NODES, D_IN]
      idx2: int32 gather indices into padded h2_full [N_PAD, D_OUT]
      dstw: f32 dst offset within the window (0..127)
      nrm:  f32 symmetric GCN norm (0 for padding slots)
    Edge slot (w, k, j) lives at [core, j, w*K + k].
    """
    src = np.asarray(edge_index[0], dtype=np.int64)
    dst = np.asarray(edge_index[1], dtype=np.int64)
    loops = np.arange(N_NODES, dtype=np.int64)
    src_all = np.concatenate([src, loops])
    dst_all = np.concatenate([dst, loops])

    deg = np.bincount(dst_all, minlength=N_NODES)
    dinv = (1.0 / np.sqrt(deg.astype(np.float64))).astype(np.float32)
    norm = dinv[src_all] * dinv[dst_all]

    core = dst_all // D_CORE
    dst_local = dst_all % D_CORE
    win = dst_local // P
    dst_in_win = (dst_local % P).astype(np.float32)
    gw = core * WINDOWS + win

    counts = np.bincount(gw, minlength=NCORES * WINDOWS)
    K = int(np.ceil(counts.max() / P))
    n_chunks = WINDOWS * K

    order = np.argsort(gw, kind="stable")
    gw_s = gw[order]
    cum = np.zeros(len(counts) + 1, np.int64)
    cum[1:] = np.cumsum(counts)
    pos = np.arange(len(gw_s), dtype=np.int64) - cum[gw_s]

    c_ = core[order]
    w_ = win[order]
    chunk = w_ * K + pos // P
    row = pos % P
    flat = c_ * (P * n_chunks) + row * n_chunks + chunk

    idx1 = np.zeros(NCORES * P * n_chunks, np.int32)
    idx2 = np.zeros(NCORES * P * n_chunks, np.int32)
    dstw = np.zeros(NCORES * P * n_chunks, np.float32)
    nrm = np.zeros(NCORES * P * n_chunks, np.float32)

    src_s = src_all[order]
    idx1[flat] = src_s
    idx2[flat] = (src_s // D_CORE) * D_PAD + (src_s % D_CORE)
    dstw[flat] = dst_in_win[order]
    nrm[flat] = norm[order]

    shape = (NCORES, P, n_chunks)
    return K, idx1.reshape(shape), idx2.reshape(shape), dstw.reshape(shape), nrm.reshape(shape)


def _build_bass(K):
    n_chunks = WINDOWS * K
    nc = bass.Bass("TRN2", num_devices=NCORES)

    x = nc.dram_tensor("x", [N_NODES, D_IN], _BF16, kind="ExternalInput")
    w1 = nc.dram_tensor("w1", [D_IN, HID], _BF16, kind="ExternalInput")
    w2a = nc.dram_tensor("w2a", [P, D_OUT], _BF16, kind="ExternalInput")
    w2b = nc.dram_tensor("w2b", [P, D_OUT], _BF16, kind="ExternalInput")
    b1 = nc.dram_tensor("b1", [1, HID], _BF16, kind="ExternalInput")
    b2 = nc.dram_tensor("b2", [1, D_OUT], _BF16, kind="ExternalInput")
    iota = nc.dram_tensor("iota", [P, P], _F32, kind="ExternalInput")
    idx1 = nc.dram_tensor("idx1", [P, n_chunks], _I32, kind="ExternalInput")
    idx2 = nc.dram_tensor("idx2", [P, n_chunks], _I32, kind="ExternalInput")
    dstw = nc.dram_tensor("dstw", [P, n_chunks], _F32, kind="ExternalInput")
    nrm = nc.dram_tensor("nrm", [P, n_chunks], _F32, kind="ExternalInput")
    out = nc.dram_tensor("out", [D_PAD, D_OUT], _F32, kind="ExternalOutput")

    h2s = nc.dram_tensor("h2s", [D_PAD, D_OUT], _BF16, kind="Internal")
    h2f = nc.dram_tensor(
        "h2f", [N_PAD, D_OUT], _BF16, kind="Internal", addr_space="Shared"
    )

    with TileContext(nc) as tc:
        with (
            tc.tile_pool(name="const", bufs=1) as cp,
            tc.tile_pool(name="work", bufs=6) as wp,
            tc.tile_pool(name="ps_acc", bufs=2, space="PSUM") as ps_acc,
            tc.tile_pool(name="ps_a", bufs=2, space="PSUM") as ps_a,
            tc.tile_pool(name="ps_b", bufs=2, space="PSUM") as ps_b,
            tc.tile_pool(name="ps_h2", bufs=2, space="PSUM") as ps_h2,
        ):
            w1_sb = cp.tile([D_IN, HID], _BF16)
            w2a_sb = cp.tile([P, D_OUT], _BF16)
            w2b_sb = cp.tile([P, D_OUT], _BF16)
            b1_sb = cp.tile([1, HID], _BF16)
            b2_sb = cp.tile([1, D_OUT], _BF16)
            iota_sb = cp.tile([P, P], _F32)
            idx1_sb = cp.tile([P, n_chunks], _I32)
            idx2_sb = cp.tile([P, n_chunks], _I32)
            dstw_sb = cp.tile([P, n_chunks], _F32)
            nrm_sb = cp.tile([P, n_chunks], _F32)
            ones_sb = cp.tile([1, P], _BF16)

            nc.sync.dma_start(out=w1_sb[:], in_=w1[:])
            nc.sync.dma_start(out=w2a_sb[:], in_=w2a[:])
            nc.sync.dma_start(out=w2b_sb[:], in_=w2b[:])
            nc.sync.dma_start(out=b1_sb[:], in_=b1[:])
            nc.sync.dma_start(out=b2_sb[:], in_=b2[:])
            nc.sync.dma_start(out=iota_sb[:], in_=iota[:])
            nc.sync.dma_start(out=idx1_sb[:], in_=idx1[:])
            nc.sync.dma_start(out=idx2_sb[:], in_=idx2[:])
            nc.sync.dma_start(out=dstw_sb[:], in_=dstw[:])
            nc.sync.dma_start(out=nrm_sb[:], in_=nrm[:])
            nc.vector.memset(ones_sb[:], 1.0)

            # ---- layer 1 + local h2 = relu(agg@W1 + b1) @ W2 ----
            for w in range(WINDOWS):
                agg_ps = ps_acc.tile([P, P], _F32, tag="acc")
                for k in range(K):
                    c = w * K + k
                    m = wp.tile([P, D_IN], _BF16, tag="m1")
                    nc.gpsimd.indirect_dma_start(
                        out=m[:],
                        out_offset=None,
                        in_=x[:],
                        in_offset=bass.IndirectOffsetOnAxis(
                            ap=idx1_sb[:, c : c + 1], axis=0
                        ),
                    )
                    s = wp.tile([P, P], _BF16, tag="s1")
                    nc.vector.tensor_scalar(
                        out=s[:],
                        in0=iota_sb[:],
                        scalar1=dstw_sb[:, c : c + 1],
                        scalar2=nrm_sb[:, c : c + 1],
                        op0=mybir.AluOpType.is_equal,
                        op1=mybir.AluOpType.mult,
                    )
                    nc.tensor.matmul(
                        out=agg_ps[:],
                        lhsT=m[:],
                        rhs=s[:],
                        start=(k == 0),
                        stop=(k == K - 1),
                    )
                agg = wp.tile([P, P], _BF16, tag="agg")
                nc.vector.tensor_copy(out=agg[:], in_=agg_ps[:])

                h1a_ps = ps_a.tile([P, P], _F32, tag="h1a")
                h1b_ps = ps_b.tile([P, P], _F32, tag="h1b")
                nc.tensor.matmul(
                    out=h1a_ps[:], lhsT=w1_sb[:, :P], rhs=agg[:], start=True, stop=False
                )
                nc.tensor.matmul(
                    out=h1a_ps[:],
                    lhsT=b1_sb[:1, :P],
                    rhs=ones_sb[:1, :],
                    start=False,
                    stop=True,
                )
                nc.tensor.matmul(
                    out=h1b_ps[:], lhsT=w1_sb[:, P:], rhs=agg[:], start=True, stop=False
                )
                nc.tensor.matmul(
                    out=h1b_ps[:],
                    lhsT=b1_sb[:1, P:],
                    rhs=ones_sb[:1, :],
                    start=False,
                    stop=True,
                )
                r1a = wp.tile([P, P], _BF16, tag="r1a")
                r1b = wp.tile([P, P], _BF16, tag="r1b")
                nc.vector.tensor_scalar_max(out=r1a[:], in0=h1a_ps[:], scalar1=0.0)
                nc.vector.tensor_scalar_max(out=r1b[:], in0=h1b_ps[:], scalar1=0.0)

                h2_ps = ps_h2.tile([P, D_OUT], _F32, tag="h2")
                nc.tensor.matmul(
                    out=h2_ps[:], lhsT=r1a[:], rhs=w2a_sb[:], start=True, stop=False
                )
                nc.tensor.matmul(
                    out=h2_ps[:], lhsT=r1b[:], rhs=w2b_sb[:], start=False, stop=True
                )
                h2w = wp.tile([P, D_OUT], _BF16, tag="h2w")
                nc.vector.tensor_copy(out=h2w[:], in_=h2_ps[:])
                nc.sync.dma_start(out=h2s[w * P : (w + 1) * P, :], in_=h2w[:])

            cc = nc.gpsimd.collective_compute(
                "AllGather",
                mybir.AluOpType.bypass,
                ins=[h2s[:]],
                outs=[h2f[:]],
                replica_groups=[list(range(NCORES))],
            )

            # ---- layer 2: out = A_hat @ h2_full + b2 ----
            for w in range(WINDOWS):
                o_ps = ps_acc.tile([P, D_OUT], _F32, tag="acc")
                nc.tensor.matmul(
                    out=o_ps[:],
                    lhsT=ones_sb[:1, :],
                    rhs=b2_sb[:1, :],
                    start=True,
                    stop=False,
                )
                for k in range(K):
                    c = w * K + k
                    m2 = wp.tile([P, D_OUT], _BF16, tag="m2")
                    g = nc.gpsimd.indirect_dma_start(
                        out=m2[:],
                        out_offset=None,
                        in_=h2f[:],
                        in_offset=bass.IndirectOffsetOnAxis(
                            ap=idx2_sb[:, c : c + 1], axis=0
                        ),
                    )
                    add_dep_helper(g.ins, cc.ins, reason="gather reads AllGather out")
                    s = wp.tile([P, P], _BF16, tag="s1")
                    nc.vector.tensor_scalar(
                        out=s[:],
                        in0=iota_sb[:],
                        scalar1=dstw_sb[:, c : c + 1],
                        scalar2=nrm_sb[:, c : c + 1],
                        op0=mybir.AluOpType.is_equal,
                        op1=mybir.AluOpType.mult,
                    )
                    nc.tensor.matmul(
                        out=o_ps[:],
                        lhsT=s[:],
                        rhs=m2[:],
                        start=False,
                        stop=(k == K - 1),
                    )
                o = wp.tile([P, D_OUT], _F32, tag="o")
                nc.vector.tensor_copy(out=o[:], in_=o_ps[:])
                nc.sync.dma_start(out=out[w * P : (w + 1) * P, :], in_=o[:])

    _split_multi_waits(nc)
    return nc


def kernel(x, edge_index, W1, b1, W2, b2):
    global LAST_EXEC_NS, LAST_RESULTS
    x = np.ascontiguousarray(np.asarray(x, dtype=np.float32).astype(bfloat16))
    W1 = np.ascontiguousarray(np.asarray(W1, dtype=np.float32).astype(bfloat16))
    W2 = np.asarray(W2, dtype=np.float32).astype(bfloat16)
    b1 = np.asarray(b1, dtype=np.float32).astype(bfloat16).reshape(1, HID)
    b2 = np.asarray(b2, dtype=np.float32).astype(bfloat16).reshape(1, D_OUT)

    K, idx1, idx2, dstw, nrm = _build_schedule(np.asarray(edge_index))
    nc = _build_bass(K)

    iota = np.tile(np.arange(P, dtype=np.float32), (P, 1))
    w2a = np.ascontiguousarray(W2[:P])
    w2b = np.ascontiguousarray(W2[P:])

    in_maps = []
    for c in range(NCORES):
        in_maps.append(
            {
                "x": x,
                "w1": W1,
                "w2a": w2a,
                "w2b": w2b,
                "b1": b1,
                "b2": b2,
                "iota": iota,
                "idx1": np.ascontiguousarray(idx1[c]),
                "idx2": np.ascontiguousarray(idx2[c]),
                "dstw": np.ascontiguousarray(dstw[c]),
                "nrm": np.ascontiguousarray(nrm[c]),
            }
        )

    res = bass_utils.run_bass_kernel_spmd(
        nc, in_maps, core_ids=list(range(NCORES)), trace=TRACE
    )
    LAST_EXEC_NS = res.exec_time_ns
    LAST_RESULTS = res

    shards = [res.results[c]["out"][:D_CORE] for c in range(NCORES)]
    return np.concatenate(shards, axis=0)



# revision 11
# speedup vs baseline: 1.0508x; 1.0508x over previous
"""2-layer GCN (PyG GCNConv x2 + ReLU) on 8 Trainium2 NeuronCores.

Strategy (graph/data parallel over destination nodes):
  - out = A_hat @ (X @ W) == (A_hat @ X) @ W  (aggregation commutes with the
    linear map), so layer 1 aggregates raw 128-dim x rows (512 B gathers) and
    layer 2 aggregates h2 = relu(out1) @ W2 rows (64-dim, 256 B gathers).
  - Each core owns a contiguous dst range of 12500 nodes (padded to
    12544 = 98 windows x 128). It processes exactly the edges whose dst lands
    in its range. Edge aggregation per 128-dst window accumulates in PSUM:
    for each 128-edge chunk, indirect-DMA gather the source rows
    (partition = edge), build the one-hot scatter matrix
    S[e, d] = norm_e * (dst_in_window_e == d) with one chained DVE
    tensor_scalar op over an iota tile, and matmul lhsT=M rhs=S accumulating
    [feat, dst] into PSUM (layer 1) / lhsT=S rhs=M2 into [dst, feat] (layer 2).
  - Between layers, one 8-rank AllGather shares each core's h2 shard
    ([12544, 64] f32, 3.2 MB) so layer-2 gathers can read any node.
  - Weights are tiny and replicated; biases enter as rank-1 matmuls into the
    same PSUM accumulation group.

Host-side preprocessing (numpy): degrees/normalization, partition edges by
dst core, sort by (window), pack into fixed-size 128-edge chunks (K chunks
per window, K = global max so the SPMD program is identical on all cores;
padding slots have norm=0 so they contribute nothing).
"""
import sys
import time

sys.path.insert(0, "/opt/trn_rl_repo")

import numpy as np
from ml_dtypes import bfloat16

import concourse.bass as bass
import concourse.mybir as mybir
from concourse.tile import TileContext
from concourse.tile_rust import add_dep_helper
from concourse import bass_utils

P = 128
N_NODES = 100000
NCORES = 8
D_CORE = N_NODES // NCORES          # 12500
WINDOWS = (D_CORE + P - 1) // P     # 98
D_PAD = WINDOWS * P                 # 12544
N_PAD = NCORES * D_PAD              # 100352
D_IN, HID, D_OUT = 128, 256, 64

# set by test.py to capture profiling info
TRACE = False
LAST_EXEC_NS = None
LAST_RESULTS = None

_F32 = mybir.dt.float32
_BF16 = mybir.dt.bfloat16
_I32 = mybir.dt.int32


def _split_multi_waits(nc):
    """walrus here refuses instructions with >1 sem wait on several ISA
    structs; split extras into standalone EventSemaphore instructions."""
    for f in nc.m.functions:
        for b in f.blocks:
            out = []
            for inst in b.instructions:
                si = inst.sync_info
                if si is not None and len(si.on_wait) > 1:
                    waits = list(si.on_wait)
                    for j, w in enumerate(waits[:-1]):
                        ev = mybir.InstEventSemaphore(
                            name=f"{inst.name}-wsplit{j}", ins=[], outs=[]
                        )
                        ev.engine = inst.engine
                        ev.sync_info = mybir.SyncInfo(on_wait=[w], on_update=[])
                        out.append(ev)
                    inst.sync_info = mybir.SyncInfo(
                        on_wait=[waits[-1]], on_update=list(si.on_update)
                    )
                out.append(inst)
            b.instructions = out


def _build_schedule(edge_index):
    """Pack edges (incl. self-loops) into per-core fixed-shape chunk arrays.

    Returns K and arrays of shape [NCORES, P, WINDOWS*K]:
      idx1: int32 gather indices into x  [N_NODES, D_IN]
      idx2: int32 gather indices into padded h2_full [N_PAD, D_OUT]
      dstw: f32 dst offset within the window (0..127)
      nrm:  f32 symmetric GCN norm (0 for padding slots)
    Edge slot (w, k, j) lives at [core, j, w*K + k].
    """
    src = np.asarray(edge_index[0], dtype=np.int64)
    dst = np.asarray(edge_index[1], dtype=np.int64)
    loops = np.arange(N_NODES, dtype=np.int64)
    src_all = np.concatenate([src, loops])
    dst_all = np.concatenate([dst, loops])

    deg = np.bincount(dst_all, minlength=N_NODES)
    dinv = (1.0 / np.sqrt(deg.astype(np.float64))).astype(np.float32)
    norm = dinv[src_all] * dinv[dst_all]

    core = dst_all // D_CORE
    dst_local = dst_all % D_CORE
    win = dst_local // P
    dst_in_win = (dst_local % P).astype(np.float32)
    gw = core * WINDOWS + win

    counts = np.bincount(gw, minlength=NCORES * WINDOWS).reshape(NCORES, WINDOWS)
    # sorted window->slot assignment per core: slot i gets each core's
    # i-th largest window, so the SPMD-uniform per-slot chunk count
    # (max over cores) wastes less padding
    perm = np.argsort(-counts, axis=1)            # slot -> window
    slotpos = np.empty_like(perm)                  # window -> slot
    for c in range(NCORES):
        slotpos[c, perm[c]] = np.arange(WINDOWS)
    csort = np.take_along_axis(counts, perm, axis=1)
    Ks = np.maximum(np.ceil(csort / P).astype(np.int64).max(axis=0), 1)  # [W]
    bases = np.zeros(WINDOWS + 1, np.int64)
    bases[1:] = np.cumsum(Ks)
    n_chunks = int(bases[-1])

    slot = slotpos[core, win]
    gs = core * WINDOWS + slot
    order = np.argsort(gs, kind="stable")
    gs_s = gs[order]
    scnt = np.bincount(gs_s, minlength=NCORES * WINDOWS)
    cum = np.zeros(len(scnt) + 1, np.int64)
    cum[1:] = np.cumsum(scnt)
    pos = np.arange(len(gs_s), dtype=np.int64) - cum[gs_s]

    c_ = core[order]
    i_ = slot[order]
    chunk = bases[i_] + pos // P
    row = pos % P
    flat = c_ * (P * n_chunks) + row * n_chunks + chunk

    idx1 = np.zeros(NCORES * P * n_chunks, np.int32)
    idx2 = np.zeros(NCORES * P * n_chunks, np.int32)
    dstw = np.zeros(NCORES * P * n_chunks, np.float32)
    nrm = np.zeros(NCORES * P * n_chunks, np.float32)

    src_s = src_all[order]
    # h2f rows are slot-major per core: row = c*D_PAD + slot*P + d
    c_s = src_s // D_CORE
    l_s = src_s % D_CORE
    slot_s = slotpos[c_s, l_s // P]
    idx1[flat] = src_s
    idx2[flat] = c_s * D_PAD + slot_s * P + (l_s % P)
    dstw[flat] = dst_in_win[order]
    nrm[flat] = norm[order]

    shape = (NCORES, P, n_chunks)
    return (
        Ks,
        bases,
        perm,
        idx1.reshape(shape),
        idx2.reshape(shape),
        dstw.reshape(shape),
        nrm.reshape(shape),
    )


def _build_bass(Ks, bases):
    n_chunks = int(bases[-1])
    nc = bass.Bass("TRN2", num_devices=NCORES)

    x = nc.dram_tensor("x", [N_NODES, D_IN], _BF16, kind="ExternalInput")
    w1 = nc.dram_tensor("w1", [D_IN, HID], _BF16, kind="ExternalInput")
    w2a = nc.dram_tensor("w2a", [P, D_OUT], _BF16, kind="ExternalInput")
    w2b = nc.dram_tensor("w2b", [P, D_OUT], _BF16, kind="ExternalInput")
    b1 = nc.dram_tensor("b1", [1, HID], _BF16, kind="ExternalInput")
    b2 = nc.dram_tensor("b2", [1, D_OUT], _BF16, kind="ExternalInput")
    iota = nc.dram_tensor("iota", [P, P], _F32, kind="ExternalInput")
    idx1 = nc.dram_tensor("idx1", [P, n_chunks], _I32, kind="ExternalInput")
    idx2 = nc.dram_tensor("idx2", [P, n_chunks], _I32, kind="ExternalInput")
    dstw = nc.dram_tensor("dstw", [P, n_chunks], _F32, kind="ExternalInput")
    nrm = nc.dram_tensor("nrm", [P, n_chunks], _F32, kind="ExternalInput")
    out = nc.dram_tensor("out", [D_PAD, D_OUT], _F32, kind="ExternalOutput")

    h2s = nc.dram_tensor("h2s", [D_PAD, D_OUT], _BF16, kind="Internal")
    h2f = nc.dram_tensor(
        "h2f", [N_PAD, D_OUT], _BF16, kind="Internal", addr_space="Shared"
    )

    with TileContext(nc) as tc:
        with (
            tc.tile_pool(name="const", bufs=1) as cp,
            tc.tile_pool(name="work", bufs=6) as wp,
            tc.tile_pool(name="ps_acc", bufs=2, space="PSUM") as ps_acc,
            tc.tile_pool(name="ps_a", bufs=2, space="PSUM") as ps_a,
            tc.tile_pool(name="ps_b", bufs=2, space="PSUM") as ps_b,
            tc.tile_pool(name="ps_h2", bufs=2, space="PSUM") as ps_h2,
        ):
            w1_sb = cp.tile([D_IN, HID], _BF16)
            w2a_sb = cp.tile([P, D_OUT], _BF16)
            w2b_sb = cp.tile([P, D_OUT], _BF16)
            b1_sb = cp.tile([1, HID], _BF16)
            b2_sb = cp.tile([1, D_OUT], _BF16)
            iota_sb = cp.tile([P, P], _F32)
            idx1_sb = cp.tile([P, n_chunks], _I32)
            idx2_sb = cp.tile([P, n_chunks], _I32)
            dstw_sb = cp.tile([P, n_chunks], _F32)
            nrm_sb = cp.tile([P, n_chunks], _F32)
            ones_sb = cp.tile([1, P], _BF16)

            nc.sync.dma_start(out=w1_sb[:], in_=w1[:])
            nc.sync.dma_start(out=w2a_sb[:], in_=w2a[:])
            nc.sync.dma_start(out=w2b_sb[:], in_=w2b[:])
            nc.sync.dma_start(out=b1_sb[:], in_=b1[:])
            nc.sync.dma_start(out=b2_sb[:], in_=b2[:])
            nc.sync.dma_start(out=iota_sb[:], in_=iota[:])
            nc.sync.dma_start(out=idx1_sb[:], in_=idx1[:])
            nc.sync.dma_start(out=idx2_sb[:], in_=idx2[:])
            nc.sync.dma_start(out=dstw_sb[:], in_=dstw[:])
            nc.sync.dma_start(out=nrm_sb[:], in_=nrm[:])
            nc.vector.memset(ones_sb[:], 1.0)

            # ---- layer 1 + local h2 = relu(agg@W1 + b1) @ W2 ----
            for w in range(WINDOWS):
                Kw = int(Ks[w])
                agg_ps = ps_acc.tile([P, P], _F32, tag="acc")
                for k in range(Kw):
                    c = int(bases[w]) + k
                    m = wp.tile([P, D_IN], _BF16, tag="m1")
                    nc.gpsimd.indirect_dma_start(
                        out=m[:],
                        out_offset=None,
                        in_=x[:],
                        in_offset=bass.IndirectOffsetOnAxis(
                            ap=idx1_sb[:, c : c + 1], axis=0
                        ),
                    )
                    s = wp.tile([P, P], _BF16, tag="s1")
                    nc.vector.tensor_scalar(
                        out=s[:],
                        in0=iota_sb[:],
                        scalar1=dstw_sb[:, c : c + 1],
                        scalar2=nrm_sb[:, c : c + 1],
                        op0=mybir.AluOpType.is_equal,
                        op1=mybir.AluOpType.mult,
                    )
                    nc.tensor.matmul(
                        out=agg_ps[:],
                        lhsT=m[:],
                        rhs=s[:],
                        start=(k == 0),
                        stop=(k == Kw - 1),
                    )
                agg = wp.tile([P, P], _BF16, tag="agg")
                nc.vector.tensor_copy(out=agg[:], in_=agg_ps[:])

                h1a_ps = ps_a.tile([P, P], _F32, tag="h1a")
                h1b_ps = ps_b.tile([P, P], _F32, tag="h1b")
                nc.tensor.matmul(
                    out=h1a_ps[:], lhsT=w1_sb[:, :P], rhs=agg[:], start=True, stop=False
                )
                nc.tensor.matmul(
                    out=h1a_ps[:],
                    lhsT=b1_sb[:1, :P],
                    rhs=ones_sb[:1, :],
                    start=False,
                    stop=True,
                )
                nc.tensor.matmul(
                    out=h1b_ps[:], lhsT=w1_sb[:, P:], rhs=agg[:], start=True, stop=False
                )
                nc.tensor.matmul(
                    out=h1b_ps[:],
                    lhsT=b1_sb[:1, P:],
                    rhs=ones_sb[:1, :],
                    start=False,
                    stop=True,
                )
                r1a = wp.tile([P, P], _BF16, tag="r1a")
                r1b = wp.tile([P, P], _BF16, tag="r1b")
                nc.vector.tensor_scalar_max(out=r1a[:], in0=h1a_ps[:], scalar1=0.0)
                nc.vector.tensor_scalar_max(out=r1b[:], in0=h1b_ps[:], scalar1=0.0)

                h2_ps = ps_h2.tile([P, D_OUT], _F32, tag="h2")
                nc.tensor.matmul(
                    out=h2_ps[:], lhsT=r1a[:], rhs=w2a_sb[:], start=True, stop=False
                )
                nc.tensor.matmul(
                    out=h2_ps[:], lhsT=r1b[:], rhs=w2b_sb[:], start=False, stop=True
                )
                h2w = wp.tile([P, D_OUT], _BF16, tag="h2w")
                nc.vector.tensor_copy(out=h2w[:], in_=h2_ps[:])
                nc.sync.dma_start(out=h2s[w * P : (w + 1) * P, :], in_=h2w[:])

            cc = nc.gpsimd.collective_compute(
                "AllGather",
                mybir.AluOpType.bypass,
                ins=[h2s[:]],
                outs=[h2f[:]],
                replica_groups=[list(range(NCORES))],
            )

            # ---- layer 2: out = A_hat @ h2_full + b2 ----
            for w in range(WINDOWS):
                Kw = int(Ks[w])
                o_ps = ps_acc.tile([P, D_OUT], _F32, tag="acc")
                nc.tensor.matmul(
                    out=o_ps[:],
                    lhsT=ones_sb[:1, :],
                    rhs=b2_sb[:1, :],
                    start=True,
                    stop=False,
                )
                for k in range(Kw):
                    c = int(bases[w]) + k
                    m2 = wp.tile([P, D_OUT], _BF16, tag="m2")
                    g = nc.gpsimd.indirect_dma_start(
                        out=m2[:],
                        out_offset=None,
                        in_=h2f[:],
                        in_offset=bass.IndirectOffsetOnAxis(
                            ap=idx2_sb[:, c : c + 1], axis=0
                        ),
                    )
                    add_dep_helper(g.ins, cc.ins, reason="gather reads AllGather out")
                    s = wp.tile([P, P], _BF16, tag="s1")
                    nc.vector.tensor_scalar(
                        out=s[:],
                        in0=iota_sb[:],
                        scalar1=dstw_sb[:, c : c + 1],
                        scalar2=nrm_sb[:, c : c + 1],
                        op0=mybir.AluOpType.is_equal,
                        op1=mybir.AluOpType.mult,
                    )
                    nc.tensor.matmul(
                        out=o_ps[:],
                        lhsT=s[:],
                        rhs=m2[:],
                        start=False,
                        stop=(k == Kw - 1),
                    )
                o = wp.tile([P, D_OUT], _F32, tag="o")
                nc.vector.tensor_copy(out=o[:], in_=o_ps[:])
                nc.sync.dma_start(out=out[w * P : (w + 1) * P, :], in_=o[:])

    _split_multi_waits(nc)
    return nc


def kernel(x, edge_index, W1, b1, W2, b2):
    global LAST_EXEC_NS, LAST_RESULTS
    x = np.ascontiguousarray(np.asarray(x, dtype=np.float32).astype(bfloat16))
    W1 = np.ascontiguousarray(np.asarray(W1, dtype=np.float32).astype(bfloat16))
    W2 = np.asarray(W2, dtype=np.float32).astype(bfloat16)
    b1 = np.asarray(b1, dtype=np.float32).astype(bfloat16).reshape(1, HID)
    b2 = np.asarray(b2, dtype=np.float32).astype(bfloat16).reshape(1, D_OUT)

    Ks, bases, perm, idx1, idx2, dstw, nrm = _build_schedule(np.asarray(edge_index))
    nc = _build_bass(Ks, bases)

    iota = np.tile(np.arange(P, dtype=np.float32), (P, 1))
    w2a = np.ascontiguousarray(W2[:P])
    w2b = np.ascontiguousarray(W2[P:])

    in_maps = []
    for c in range(NCORES):
        in_maps.append(
            {
                "x": x,
                "w1": W1,
                "w2a": w2a,
                "w2b": w2b,
                "b1": b1,
                "b2": b2,
                "iota": iota,
                "idx1": np.ascontiguousarray(idx1[c]),
                "idx2": np.ascontiguousarray(idx2[c]),
                "dstw": np.ascontiguousarray(dstw[c]),
                "nrm": np.ascontiguousarray(nrm[c]),
            }
        )

    res = bass_utils.run_bass_kernel_spmd(
        nc, in_maps, core_ids=list(range(NCORES)), trace=TRACE
    )
    LAST_EXEC_NS = res.exec_time_ns
    LAST_RESULTS = res

    # un-permute: out row slot*P + d -> node perm[c][slot]*P + d
    shards = []
    for c in range(NCORES):
        a = res.results[c]["out"].reshape(WINDOWS, P, D_OUT)
        inv = np.empty(WINDOWS, np.int64)
        inv[perm[c]] = np.arange(WINDOWS)
        shards.append(a[inv].reshape(D_PAD, D_OUT)[:D_CORE])
    return np.concatenate(shards, axis=0)


# revision 12
# speedup vs baseline: 1.1666x; 1.1102x over previous
"""2-layer GCN (PyG GCNConv x2 + ReLU) on 8 Trainium2 NeuronCores.

Strategy (graph/data parallel over destination nodes):
  - out = A_hat @ (X @ W) == (A_hat @ X) @ W  (aggregation commutes with the
    linear map), so layer 1 aggregates raw 128-dim x rows (512 B gathers) and
    layer 2 aggregates h2 = relu(out1) @ W2 rows (64-dim, 256 B gathers).
  - Each core owns a contiguous dst range of 12500 nodes (padded to
    12544 = 98 windows x 128). It processes exactly the edges whose dst lands
    in its range. Edge aggregation per 128-dst window accumulates in PSUM:
    for each 128-edge chunk, indirect-DMA gather the source rows
    (partition = edge), build the one-hot scatter matrix
    S[e, d] = norm_e * (dst_in_window_e == d) with one chained DVE
    tensor_scalar op over an iota tile, and matmul lhsT=M rhs=S accumulating
    [feat, dst] into PSUM (layer 1) / lhsT=S rhs=M2 into [dst, feat] (layer 2).
  - Between layers, one 8-rank AllGather shares each core's h2 shard
    ([12544, 64] f32, 3.2 MB) so layer-2 gathers can read any node.
  - Weights are tiny and replicated; biases enter as rank-1 matmuls into the
    same PSUM accumulation group.

Host-side preprocessing (numpy): degrees/normalization, partition edges by
dst core, sort by (window), pack into fixed-size 128-edge chunks (K chunks
per window, K = global max so the SPMD program is identical on all cores;
padding slots have norm=0 so they contribute nothing).
"""
import sys
import time

sys.path.insert(0, "/opt/trn_rl_repo")

import numpy as np
from ml_dtypes import bfloat16

import concourse.bass as bass
import concourse.mybir as mybir
from concourse.tile import TileContext
from concourse.tile_rust import add_dep_helper
from concourse import bass_utils

P = 128
N_NODES = 100000
NCORES = 8
D_CORE = N_NODES // NCORES          # 12500
WINDOWS = (D_CORE + P - 1) // P     # 98
D_PAD = WINDOWS * P                 # 12544
N_PAD = NCORES * D_PAD              # 100352
D_IN, HID, D_OUT = 128, 256, 64

# set by test.py to capture profiling info
TRACE = False
LAST_EXEC_NS = None
LAST_RESULTS = None

_F32 = mybir.dt.float32
_BF16 = mybir.dt.bfloat16
_I32 = mybir.dt.int32


def _split_multi_waits(nc):
    """walrus here refuses instructions with >1 sem wait on several ISA
    structs; split extras into standalone EventSemaphore instructions."""
    for f in nc.m.functions:
        for b in f.blocks:
            out = []
            for inst in b.instructions:
                si = inst.sync_info
                if si is not None and len(si.on_wait) > 1:
                    waits = list(si.on_wait)
                    for j, w in enumerate(waits[:-1]):
                        ev = mybir.InstEventSemaphore(
                            name=f"{inst.name}-wsplit{j}", ins=[], outs=[]
                        )
                        ev.engine = inst.engine
                        ev.sync_info = mybir.SyncInfo(on_wait=[w], on_update=[])
                        out.append(ev)
                    inst.sync_info = mybir.SyncInfo(
                        on_wait=[waits[-1]], on_update=list(si.on_update)
                    )
                out.append(inst)
            b.instructions = out


def _build_schedule(edge_index):
    """Pack edges (incl. self-loops) into per-core fixed-shape chunk arrays.

    Returns K and arrays of shape [NCORES, P, WINDOWS*K]:
      idx1: int32 gather indices into x  [N_NODES, D_IN]
      idx2: int32 gather indices into padded h2_full [N_PAD, D_OUT]
      dstw: f32 dst offset within the window (0..127)
      nrm:  f32 symmetric GCN norm (0 for padding slots)
    Edge slot (w, k, j) lives at [core, j, w*K + k].
    """
    src = np.asarray(edge_index[0], dtype=np.int64)
    dst = np.asarray(edge_index[1], dtype=np.int64)
    loops = np.arange(N_NODES, dtype=np.int64)
    # degree includes the self-loops, but the loops themselves are NOT
    # packed as gather slots: their x rows are contiguous per window, so
    # the kernel adds them via a plain HWDGE load + diagonal matmul
    deg = np.bincount(np.concatenate([dst, loops]), minlength=N_NODES)
    dinv = (1.0 / np.sqrt(deg.astype(np.float64))).astype(np.float32)
    src_all = src
    dst_all = dst
    norm = dinv[src_all] * dinv[dst_all]

    core = dst_all // D_CORE
    dst_local = dst_all % D_CORE
    win = dst_local // P
    dst_in_win = (dst_local % P).astype(np.float32)
    gw = core * WINDOWS + win

    counts = np.bincount(gw, minlength=NCORES * WINDOWS).reshape(NCORES, WINDOWS)
    # sorted window->slot assignment per core: slot i gets each core's
    # i-th largest window, so the SPMD-uniform per-slot chunk count
    # (max over cores) wastes less padding
    perm = np.argsort(-counts, axis=1)            # slot -> window
    slotpos = np.empty_like(perm)                  # window -> slot
    for c in range(NCORES):
        slotpos[c, perm[c]] = np.arange(WINDOWS)
    csort = np.take_along_axis(counts, perm, axis=1)
    Ks = np.maximum(np.ceil(csort / P).astype(np.int64).max(axis=0), 1)  # [W]
    bases = np.zeros(WINDOWS + 1, np.int64)
    bases[1:] = np.cumsum(Ks)
    n_chunks = int(bases[-1])

    slot = slotpos[core, win]
    gs = core * WINDOWS + slot
    order = np.argsort(gs, kind="stable")
    gs_s = gs[order]
    scnt = np.bincount(gs_s, minlength=NCORES * WINDOWS)
    cum = np.zeros(len(scnt) + 1, np.int64)
    cum[1:] = np.cumsum(scnt)
    pos = np.arange(len(gs_s), dtype=np.int64) - cum[gs_s]

    c_ = core[order]
    i_ = slot[order]
    chunk = bases[i_] + pos // P
    row = pos % P
    flat = c_ * (P * n_chunks) + row * n_chunks + chunk

    idx1 = np.zeros(NCORES * P * n_chunks, np.int32)
    idx2 = np.zeros(NCORES * P * n_chunks, np.int32)
    dstw = np.zeros(NCORES * P * n_chunks, np.float32)
    nrm = np.zeros(NCORES * P * n_chunks, np.float32)

    src_s = src_all[order]
    # h2f rows are slot-major per core: row = c*D_PAD + slot*P + d
    c_s = src_s // D_CORE
    l_s = src_s % D_CORE
    slot_s = slotpos[c_s, l_s // P]
    idx1[flat] = src_s
    idx2[flat] = c_s * D_PAD + slot_s * P + (l_s % P)
    dstw[flat] = dst_in_win[order]
    nrm[flat] = norm[order]

    shape = (NCORES, P, n_chunks)
    return (
        Ks,
        bases,
        perm,
        dinv,
        idx1.reshape(shape),
        idx2.reshape(shape),
        dstw.reshape(shape),
        nrm.reshape(shape),
    )


def _build_bass(Ks, bases):
    n_chunks = int(bases[-1])
    nc = bass.Bass("TRN2", num_devices=NCORES)

    x = nc.dram_tensor("x", [N_NODES, D_IN], _BF16, kind="ExternalInput")
    w1 = nc.dram_tensor("w1", [D_IN, HID], _BF16, kind="ExternalInput")
    w2a = nc.dram_tensor("w2a", [P, D_OUT], _BF16, kind="ExternalInput")
    w2b = nc.dram_tensor("w2b", [P, D_OUT], _BF16, kind="ExternalInput")
    b1 = nc.dram_tensor("b1", [1, HID], _BF16, kind="ExternalInput")
    b2 = nc.dram_tensor("b2", [1, D_OUT], _BF16, kind="ExternalInput")
    iota = nc.dram_tensor("iota", [P, P], _F32, kind="ExternalInput")
    idx1 = nc.dram_tensor("idx1", [P, n_chunks], _I32, kind="ExternalInput")
    idx2 = nc.dram_tensor("idx2", [P, n_chunks], _I32, kind="ExternalInput")
    dstw = nc.dram_tensor("dstw", [P, n_chunks], _F32, kind="ExternalInput")
    nrm = nc.dram_tensor("nrm", [P, n_chunks], _F32, kind="ExternalInput")
    # per-core slot-ordered own x rows + squared self-loop norms
    xloc = nc.dram_tensor("xloc", [D_PAD, D_IN], _BF16, kind="ExternalInput")
    dinv2 = nc.dram_tensor("dinv2", [P, WINDOWS], _F32, kind="ExternalInput")
    iotac = nc.dram_tensor("iotac", [P, 1], _F32, kind="ExternalInput")
    out = nc.dram_tensor("out", [D_PAD, D_OUT], _F32, kind="ExternalOutput")

    h2s = nc.dram_tensor("h2s", [D_PAD, D_OUT], _BF16, kind="Internal")
    h2f = nc.dram_tensor(
        "h2f", [N_PAD, D_OUT], _BF16, kind="Internal", addr_space="Shared"
    )

    with TileContext(nc) as tc:
        with (
            tc.tile_pool(name="const", bufs=1) as cp,
            tc.tile_pool(name="work", bufs=6) as wp,
            tc.tile_pool(name="ps_acc", bufs=2, space="PSUM") as ps_acc,
            tc.tile_pool(name="ps_a", bufs=2, space="PSUM") as ps_a,
            tc.tile_pool(name="ps_b", bufs=2, space="PSUM") as ps_b,
            tc.tile_pool(name="ps_h2", bufs=2, space="PSUM") as ps_h2,
        ):
            w1_sb = cp.tile([D_IN, HID], _BF16)
            w2a_sb = cp.tile([P, D_OUT], _BF16)
            w2b_sb = cp.tile([P, D_OUT], _BF16)
            b1_sb = cp.tile([1, HID], _BF16)
            b2_sb = cp.tile([1, D_OUT], _BF16)
            iota_sb = cp.tile([P, P], _F32)
            idx1_sb = cp.tile([P, n_chunks], _I32)
            idx2_sb = cp.tile([P, n_chunks], _I32)
            dstw_sb = cp.tile([P, n_chunks], _F32)
            nrm_sb = cp.tile([P, n_chunks], _F32)
            dinv2_sb = cp.tile([P, WINDOWS], _F32)
            iotac_sb = cp.tile([P, 1], _F32)
            ones_sb = cp.tile([1, P], _BF16)

            nc.sync.dma_start(out=w1_sb[:], in_=w1[:])
            nc.sync.dma_start(out=w2a_sb[:], in_=w2a[:])
            nc.sync.dma_start(out=w2b_sb[:], in_=w2b[:])
            nc.sync.dma_start(out=b1_sb[:], in_=b1[:])
            nc.sync.dma_start(out=b2_sb[:], in_=b2[:])
            nc.sync.dma_start(out=iota_sb[:], in_=iota[:])
            nc.sync.dma_start(out=idx1_sb[:], in_=idx1[:])
            nc.sync.dma_start(out=idx2_sb[:], in_=idx2[:])
            nc.sync.dma_start(out=dstw_sb[:], in_=dstw[:])
            nc.sync.dma_start(out=nrm_sb[:], in_=nrm[:])
            nc.sync.dma_start(out=dinv2_sb[:], in_=dinv2[:])
            nc.sync.dma_start(out=iotac_sb[:], in_=iotac[:])
            nc.vector.memset(ones_sb[:], 1.0)

            def build_diag(w):
                dg = wp.tile([P, P], _BF16, tag="dg")
                nc.vector.tensor_scalar(
                    out=dg[:],
                    in0=iota_sb[:],
                    scalar1=iotac_sb[:, 0:1],
                    scalar2=dinv2_sb[:, w : w + 1],
                    op0=mybir.AluOpType.is_equal,
                    op1=mybir.AluOpType.mult,
                )
                return dg

            # ---- layer 1 + local h2 = relu(agg@W1 + b1) @ W2 ----
            h2_writes = [None] * WINDOWS
            for w in range(WINDOWS):
                Kw = int(Ks[w])
                agg_ps = ps_acc.tile([P, P], _F32, tag="acc")
                xw = wp.tile([P, D_IN], _BF16, tag="xw")
                nc.sync.dma_start(out=xw[:], in_=xloc[w * P : (w + 1) * P, :])
                dg = build_diag(w)
                nc.tensor.matmul(
                    out=agg_ps[:], lhsT=xw[:], rhs=dg[:], start=True, stop=False
                )
                for k in range(Kw):
                    c = int(bases[w]) + k
                    m = wp.tile([P, D_IN], _BF16, tag="m1")
                    nc.gpsimd.indirect_dma_start(
                        out=m[:],
                        out_offset=None,
                        in_=x[:],
                        in_offset=bass.IndirectOffsetOnAxis(
                            ap=idx1_sb[:, c : c + 1], axis=0
                        ),
                    )
                    s = wp.tile([P, P], _BF16, tag="s1")
                    nc.vector.tensor_scalar(
                        out=s[:],
                        in0=iota_sb[:],
                        scalar1=dstw_sb[:, c : c + 1],
                        scalar2=nrm_sb[:, c : c + 1],
                        op0=mybir.AluOpType.is_equal,
                        op1=mybir.AluOpType.mult,
                    )
                    nc.tensor.matmul(
                        out=agg_ps[:],
                        lhsT=m[:],
                        rhs=s[:],
                        start=False,
                        stop=(k == Kw - 1),
                    )
                agg = wp.tile([P, P], _BF16, tag="agg")
                nc.vector.tensor_copy(out=agg[:], in_=agg_ps[:])

                h1a_ps = ps_a.tile([P, P], _F32, tag="h1a")
                h1b_ps = ps_b.tile([P, P], _F32, tag="h1b")
                nc.tensor.matmul(
                    out=h1a_ps[:], lhsT=w1_sb[:, :P], rhs=agg[:], start=True, stop=False
                )
                nc.tensor.matmul(
                    out=h1a_ps[:],
                    lhsT=b1_sb[:1, :P],
                    rhs=ones_sb[:1, :],
                    start=False,
                    stop=True,
                )
                nc.tensor.matmul(
                    out=h1b_ps[:], lhsT=w1_sb[:, P:], rhs=agg[:], start=True, stop=False
                )
                nc.tensor.matmul(
                    out=h1b_ps[:],
                    lhsT=b1_sb[:1, P:],
                    rhs=ones_sb[:1, :],
                    start=False,
                    stop=True,
                )
                r1a = wp.tile([P, P], _BF16, tag="r1a")
                r1b = wp.tile([P, P], _BF16, tag="r1b")
                nc.vector.tensor_scalar_max(out=r1a[:], in0=h1a_ps[:], scalar1=0.0)
                nc.vector.tensor_scalar_max(out=r1b[:], in0=h1b_ps[:], scalar1=0.0)

                h2_ps = ps_h2.tile([P, D_OUT], _F32, tag="h2")
                nc.tensor.matmul(
                    out=h2_ps[:], lhsT=r1a[:], rhs=w2a_sb[:], start=True, stop=False
                )
                nc.tensor.matmul(
                    out=h2_ps[:], lhsT=r1b[:], rhs=w2b_sb[:], start=False, stop=True
                )
                h2w = wp.tile([P, D_OUT], _BF16, tag="h2w")
                nc.vector.tensor_copy(out=h2w[:], in_=h2_ps[:])
                h2_writes[w] = nc.sync.dma_start(
                    out=h2s[w * P : (w + 1) * P, :], in_=h2w[:]
                )

            cc = nc.gpsimd.collective_compute(
                "AllGather",
                mybir.AluOpType.bypass,
                ins=[h2s[:]],
                outs=[h2f[:]],
                replica_groups=[list(range(NCORES))],
            )

            # ---- layer 2: out = A_hat @ h2_full + b2 ----
            for w in range(WINDOWS):
                Kw = int(Ks[w])
                o_ps = ps_acc.tile([P, D_OUT], _F32, tag="acc")
                h2loc = wp.tile([P, D_OUT], _BF16, tag="h2loc")
                rb = nc.sync.dma_start(
                    out=h2loc[:], in_=h2s[w * P : (w + 1) * P, :]
                )
                add_dep_helper(
                    rb.ins, h2_writes[w].ins, reason="read back own h2 piece"
                )
                dg = build_diag(w)
                nc.tensor.matmul(
                    out=o_ps[:],
                    lhsT=ones_sb[:1, :],
                    rhs=b2_sb[:1, :],
                    start=True,
                    stop=False,
                )
                nc.tensor.matmul(
                    out=o_ps[:], lhsT=dg[:], rhs=h2loc[:], start=False, stop=False
                )
                for k in range(Kw):
                    c = int(bases[w]) + k
                    m2 = wp.tile([P, D_OUT], _BF16, tag="m2")
                    g = nc.gpsimd.indirect_dma_start(
                        out=m2[:],
                        out_offset=None,
                        in_=h2f[:],
                        in_offset=bass.IndirectOffsetOnAxis(
                            ap=idx2_sb[:, c : c + 1], axis=0
                        ),
                    )
                    add_dep_helper(g.ins, cc.ins, reason="gather reads AllGather out")
                    s = wp.tile([P, P], _BF16, tag="s1")
                    nc.vector.tensor_scalar(
                        out=s[:],
                        in0=iota_sb[:],
                        scalar1=dstw_sb[:, c : c + 1],
                        scalar2=nrm_sb[:, c : c + 1],
                        op0=mybir.AluOpType.is_equal,
                        op1=mybir.AluOpType.mult,
                    )
                    nc.tensor.matmul(
                        out=o_ps[:],
                        lhsT=s[:],
                        rhs=m2[:],
                        start=False,
                        stop=(k == Kw - 1),
                    )
                o = wp.tile([P, D_OUT], _F32, tag="o")
                nc.vector.tensor_copy(out=o[:], in_=o_ps[:])
                nc.sync.dma_start(out=out[w * P : (w + 1) * P, :], in_=o[:])

    _split_multi_waits(nc)
    return nc


def kernel(x, edge_index, W1, b1, W2, b2):
    global LAST_EXEC_NS, LAST_RESULTS
    x = np.ascontiguousarray(np.asarray(x, dtype=np.float32).astype(bfloat16))
    W1 = np.ascontiguousarray(np.asarray(W1, dtype=np.float32).astype(bfloat16))
    W2 = np.asarray(W2, dtype=np.float32).astype(bfloat16)
    b1 = np.asarray(b1, dtype=np.float32).astype(bfloat16).reshape(1, HID)
    b2 = np.asarray(b2, dtype=np.float32).astype(bfloat16).reshape(1, D_OUT)

    Ks, bases, perm, dinv, idx1, idx2, dstw, nrm = _build_schedule(
        np.asarray(edge_index)
    )
    nc = _build_bass(Ks, bases)

    iota = np.tile(np.arange(P, dtype=np.float32), (P, 1))
    iotac = np.arange(P, dtype=np.float32).reshape(P, 1)
    w2a = np.ascontiguousarray(W2[:P])
    w2b = np.ascontiguousarray(W2[P:])

    # per-core slot-ordered x rows + squared dinv (0 on padded tail rows)
    xlocs, dinv2s = [], []
    for c in range(NCORES):
        nodes = (
            c * D_CORE
            + perm[c][:, None] * P
            + np.arange(P)[None, :]
        ).reshape(-1)
        valid = nodes < (c + 1) * D_CORE
        nv = np.where(valid, nodes, 0)
        xl = np.where(valid[:, None], x[nv].astype(np.float32), 0.0).astype(
            bfloat16
        )
        d2 = np.where(valid, dinv[nv] ** 2, 0.0).astype(np.float32)
        xlocs.append(np.ascontiguousarray(xl))
        # dinv2 layout [P, WINDOWS]: partition r, slot w
        dinv2s.append(np.ascontiguousarray(d2.reshape(WINDOWS, P).T))

    in_maps = []
    for c in range(NCORES):
        in_maps.append(
            {
                "x": x,
                "w1": W1,
                "w2a": w2a,
                "w2b": w2b,
                "b1": b1,
                "b2": b2,
                "iota": iota,
                "xloc": xlocs[c],
                "dinv2": dinv2s[c],
                "iotac": iotac,
                "idx1": np.ascontiguousarray(idx1[c]),
                "idx2": np.ascontiguousarray(idx2[c]),
                "dstw": np.ascontiguousarray(dstw[c]),
                "nrm": np.ascontiguousarray(nrm[c]),
            }
        )

    res = bass_utils.run_bass_kernel_spmd(
        nc, in_maps, core_ids=list(range(NCORES)), trace=TRACE
    )
    LAST_EXEC_NS = res.exec_time_ns
    LAST_RESULTS = res

    # un-permute: out row slot*P + d -> node perm[c][slot]*P + d
    shards = []
    for c in range(NCORES):
        a = res.results[c]["out"].reshape(WINDOWS, P, D_OUT)
        inv = np.empty(WINDOWS, np.int64)
        inv[perm[c]] = np.arange(WINDOWS)
        shards.append(a[inv].reshape(D_PAD, D_OUT)[:D_CORE])
    return np.concatenate(shards, axis=0)


# revision 13
# speedup vs baseline: 1.1752x; 1.0074x over previous
"""2-layer GCN (PyG GCNConv x2 + ReLU) on 8 Trainium2 NeuronCores.

Strategy (graph/data parallel over destination nodes):
  - out = A_hat @ (X @ W) == (A_hat @ X) @ W  (aggregation commutes with the
    linear map), so layer 1 aggregates raw 128-dim x rows (512 B gathers) and
    layer 2 aggregates h2 = relu(out1) @ W2 rows (64-dim, 256 B gathers).
  - Each core owns a contiguous dst range of 12500 nodes (padded to
    12544 = 98 windows x 128). It processes exactly the edges whose dst lands
    in its range. Edge aggregation per 128-dst window accumulates in PSUM:
    for each 128-edge chunk, indirect-DMA gather the source rows
    (partition = edge), build the one-hot scatter matrix
    S[e, d] = norm_e * (dst_in_window_e == d) with one chained DVE
    tensor_scalar op over an iota tile, and matmul lhsT=M rhs=S accumulating
    [feat, dst] into PSUM (layer 1) / lhsT=S rhs=M2 into [dst, feat] (layer 2).
  - Between layers, one 8-rank AllGather shares each core's h2 shard
    ([12544, 64] f32, 3.2 MB) so layer-2 gathers can read any node.
  - Weights are tiny and replicated; biases enter as rank-1 matmuls into the
    same PSUM accumulation group.

Host-side preprocessing (numpy): degrees/normalization, partition edges by
dst core, sort by (window), pack into fixed-size 128-edge chunks (K chunks
per window, K = global max so the SPMD program is identical on all cores;
padding slots have norm=0 so they contribute nothing).
"""
import sys
import time

sys.path.insert(0, "/opt/trn_rl_repo")

import numpy as np
from ml_dtypes import bfloat16

import concourse.bass as bass
import concourse.mybir as mybir
from concourse.tile import TileContext
from concourse.tile_rust import add_dep_helper
from concourse import bass_utils

P = 128
N_NODES = 100000
NCORES = 8
D_CORE = N_NODES // NCORES          # 12500
WINDOWS = (D_CORE + P - 1) // P     # 98
D_PAD = WINDOWS * P                 # 12544
N_PAD = NCORES * D_PAD              # 100352
D_IN, HID, D_OUT = 128, 256, 64

# set by test.py to capture profiling info
TRACE = False
LAST_EXEC_NS = None
LAST_RESULTS = None

_F32 = mybir.dt.float32
_BF16 = mybir.dt.bfloat16
_I32 = mybir.dt.int32


def _split_multi_waits(nc):
    """walrus here refuses instructions with >1 sem wait on several ISA
    structs; split extras into standalone EventSemaphore instructions."""
    for f in nc.m.functions:
        for b in f.blocks:
            out = []
            for inst in b.instructions:
                si = inst.sync_info
                if si is not None and len(si.on_wait) > 1:
                    waits = list(si.on_wait)
                    for j, w in enumerate(waits[:-1]):
                        ev = mybir.InstEventSemaphore(
                            name=f"{inst.name}-wsplit{j}", ins=[], outs=[]
                        )
                        ev.engine = inst.engine
                        ev.sync_info = mybir.SyncInfo(on_wait=[w], on_update=[])
                        out.append(ev)
                    inst.sync_info = mybir.SyncInfo(
                        on_wait=[waits[-1]], on_update=list(si.on_update)
                    )
                out.append(inst)
            b.instructions = out


def _build_schedule(edge_index):
    """Pack edges (incl. self-loops) into per-core fixed-shape chunk arrays.

    Returns K and arrays of shape [NCORES, P, WINDOWS*K]:
      idx1: int32 gather indices into x  [N_NODES, D_IN]
      idx2: int32 gather indices into padded h2_full [N_PAD, D_OUT]
      dstw: f32 dst offset within the window (0..127)
      nrm:  f32 symmetric GCN norm (0 for padding slots)
    Edge slot (w, k, j) lives at [core, j, w*K + k].
    """
    src = np.asarray(edge_index[0], dtype=np.int64)
    dst = np.asarray(edge_index[1], dtype=np.int64)
    loops = np.arange(N_NODES, dtype=np.int64)
    # degree includes the self-loops, but the loops themselves are NOT
    # packed as gather slots: their x rows are contiguous per window, so
    # the kernel adds them via a plain HWDGE load + diagonal matmul
    deg = np.bincount(np.concatenate([dst, loops]), minlength=N_NODES)
    dinv = (1.0 / np.sqrt(deg.astype(np.float64))).astype(np.float32)
    src_all = src
    dst_all = dst
    norm = dinv[src_all] * dinv[dst_all]

    core = dst_all // D_CORE
    dst_local = dst_all % D_CORE
    win = dst_local // P
    dst_in_win = (dst_local % P).astype(np.float32)
    gw = core * WINDOWS + win

    counts = np.bincount(gw, minlength=NCORES * WINDOWS).reshape(NCORES, WINDOWS)
    # sorted window->slot assignment per core: slot i gets each core's
    # i-th largest window, so the SPMD-uniform per-slot chunk count
    # (max over cores) wastes less padding
    perm = np.argsort(-counts, axis=1)            # slot -> window
    slotpos = np.empty_like(perm)                  # window -> slot
    for c in range(NCORES):
        slotpos[c, perm[c]] = np.arange(WINDOWS)
    csort = np.take_along_axis(counts, perm, axis=1)
    Ks = np.maximum(np.ceil(csort / P).astype(np.int64).max(axis=0), 1)  # [W]
    bases = np.zeros(WINDOWS + 1, np.int64)
    bases[1:] = np.cumsum(Ks)
    n_chunks = int(bases[-1])

    slot = slotpos[core, win]
    gs = core * WINDOWS + slot
    order = np.argsort(gs, kind="stable")
    gs_s = gs[order]
    scnt = np.bincount(gs_s, minlength=NCORES * WINDOWS)
    cum = np.zeros(len(scnt) + 1, np.int64)
    cum[1:] = np.cumsum(scnt)
    pos = np.arange(len(gs_s), dtype=np.int64) - cum[gs_s]

    c_ = core[order]
    i_ = slot[order]
    chunk = bases[i_] + pos // P
    row = pos % P
    flat = c_ * (P * n_chunks) + row * n_chunks + chunk

    idx1 = np.zeros(NCORES * P * n_chunks, np.int32)
    idx2 = np.zeros(NCORES * P * n_chunks, np.int32)
    dstw = np.zeros(NCORES * P * n_chunks, np.float32)
    nrm = np.zeros(NCORES * P * n_chunks, np.float32)

    src_s = src_all[order]
    # h2f rows are slot-major per core: row = c*D_PAD + slot*P + d
    c_s = src_s // D_CORE
    l_s = src_s % D_CORE
    slot_s = slotpos[c_s, l_s // P]
    idx1[flat] = src_s
    idx2[flat] = c_s * D_PAD + slot_s * P + (l_s % P)
    dstw[flat] = dst_in_win[order]
    nrm[flat] = norm[order]

    shape = (NCORES, P, n_chunks)
    return (
        Ks,
        bases,
        perm,
        dinv,
        idx1.reshape(shape),
        idx2.reshape(shape),
        dstw.reshape(shape),
        nrm.reshape(shape),
    )


def _build_bass(Ks, bases):
    n_chunks = int(bases[-1])
    nc = bass.Bass("TRN2", num_devices=NCORES)

    x = nc.dram_tensor("x", [N_NODES, D_IN], _BF16, kind="ExternalInput")
    w1 = nc.dram_tensor("w1", [D_IN, HID], _BF16, kind="ExternalInput")
    w2a = nc.dram_tensor("w2a", [P, D_OUT], _BF16, kind="ExternalInput")
    w2b = nc.dram_tensor("w2b", [P, D_OUT], _BF16, kind="ExternalInput")
    b1 = nc.dram_tensor("b1", [1, HID], _BF16, kind="ExternalInput")
    b2 = nc.dram_tensor("b2", [1, D_OUT], _BF16, kind="ExternalInput")
    iota = nc.dram_tensor("iota", [P, P], _F32, kind="ExternalInput")
    idx1 = nc.dram_tensor("idx1", [P, n_chunks], _I32, kind="ExternalInput")
    idx2 = nc.dram_tensor("idx2", [P, n_chunks], _I32, kind="ExternalInput")
    dstw = nc.dram_tensor("dstw", [P, n_chunks], _F32, kind="ExternalInput")
    nrm = nc.dram_tensor("nrm", [P, n_chunks], _F32, kind="ExternalInput")
    # per-core slot-ordered own x rows + squared self-loop norms
    xloc = nc.dram_tensor("xloc", [D_PAD, D_IN], _BF16, kind="ExternalInput")
    dinv2 = nc.dram_tensor("dinv2", [P, WINDOWS], _F32, kind="ExternalInput")
    iotac = nc.dram_tensor("iotac", [P, 1], _F32, kind="ExternalInput")
    out = nc.dram_tensor("out", [D_PAD, D_OUT], _F32, kind="ExternalOutput")

    h2s = nc.dram_tensor("h2s", [D_PAD, D_OUT], _BF16, kind="Internal")
    h2f = nc.dram_tensor(
        "h2f", [N_PAD, D_OUT], _BF16, kind="Internal", addr_space="Shared"
    )

    with TileContext(nc) as tc:
        with (
            tc.tile_pool(name="const", bufs=1) as cp,
            tc.tile_pool(name="work", bufs=12) as wp,
            tc.tile_pool(name="ps_acc", bufs=2, space="PSUM") as ps_acc,
            tc.tile_pool(name="ps_a", bufs=2, space="PSUM") as ps_a,
            tc.tile_pool(name="ps_b", bufs=2, space="PSUM") as ps_b,
            tc.tile_pool(name="ps_h2", bufs=2, space="PSUM") as ps_h2,
        ):
            w1_sb = cp.tile([D_IN, HID], _BF16)
            w2a_sb = cp.tile([P, D_OUT], _BF16)
            w2b_sb = cp.tile([P, D_OUT], _BF16)
            b1_sb = cp.tile([1, HID], _BF16)
            b2_sb = cp.tile([1, D_OUT], _BF16)
            iota_sb = cp.tile([P, P], _F32)
            idx1_sb = cp.tile([P, n_chunks], _I32)
            idx2_sb = cp.tile([P, n_chunks], _I32)
            dstw_sb = cp.tile([P, n_chunks], _F32)
            nrm_sb = cp.tile([P, n_chunks], _F32)
            dinv2_sb = cp.tile([P, WINDOWS], _F32)
            iotac_sb = cp.tile([P, 1], _F32)
            ones_sb = cp.tile([1, P], _BF16)

            nc.sync.dma_start(out=idx1_sb[:], in_=idx1[:])
            nc.sync.dma_start(out=idx2_sb[:], in_=idx2[:])
            nc.sync.dma_start(out=w1_sb[:], in_=w1[:])
            nc.sync.dma_start(out=w2a_sb[:], in_=w2a[:])
            nc.sync.dma_start(out=w2b_sb[:], in_=w2b[:])
            nc.sync.dma_start(out=b1_sb[:], in_=b1[:])
            nc.sync.dma_start(out=b2_sb[:], in_=b2[:])
            nc.sync.dma_start(out=iota_sb[:], in_=iota[:])
            nc.sync.dma_start(out=dstw_sb[:], in_=dstw[:])
            nc.sync.dma_start(out=nrm_sb[:], in_=nrm[:])
            nc.sync.dma_start(out=dinv2_sb[:], in_=dinv2[:])
            nc.sync.dma_start(out=iotac_sb[:], in_=iotac[:])
            nc.vector.memset(ones_sb[:], 1.0)

            def build_diag(w):
                dg = wp.tile([P, P], _BF16, tag="dg")
                nc.vector.tensor_scalar(
                    out=dg[:],
                    in0=iota_sb[:],
                    scalar1=iotac_sb[:, 0:1],
                    scalar2=dinv2_sb[:, w : w + 1],
                    op0=mybir.AluOpType.is_equal,
                    op1=mybir.AluOpType.mult,
                )
                return dg

            # ---- layer 1 + local h2 = relu(agg@W1 + b1) @ W2 ----
            h2_writes = [None] * WINDOWS
            for w in range(WINDOWS):
                Kw = int(Ks[w])
                agg_ps = ps_acc.tile([P, P], _F32, tag="acc")
                xw = wp.tile([P, D_IN], _BF16, tag="xw")
                nc.sync.dma_start(out=xw[:], in_=xloc[w * P : (w + 1) * P, :])
                dg = build_diag(w)
                nc.tensor.matmul(
                    out=agg_ps[:], lhsT=xw[:], rhs=dg[:], start=True, stop=False
                )
                for k in range(Kw):
                    c = int(bases[w]) + k
                    m = wp.tile([P, D_IN], _BF16, tag="m1")
                    nc.gpsimd.indirect_dma_start(
                        out=m[:],
                        out_offset=None,
                        in_=x[:],
                        in_offset=bass.IndirectOffsetOnAxis(
                            ap=idx1_sb[:, c : c + 1], axis=0
                        ),
                    )
                    s = wp.tile([P, P], _BF16, tag="s1")
                    nc.vector.tensor_scalar(
                        out=s[:],
                        in0=iota_sb[:],
                        scalar1=dstw_sb[:, c : c + 1],
                        scalar2=nrm_sb[:, c : c + 1],
                        op0=mybir.AluOpType.is_equal,
                        op1=mybir.AluOpType.mult,
                    )
                    nc.tensor.matmul(
                        out=agg_ps[:],
                        lhsT=m[:],
                        rhs=s[:],
                        start=False,
                        stop=(k == Kw - 1),
                    )
                agg = wp.tile([P, P], _BF16, tag="agg")
                nc.vector.tensor_copy(out=agg[:], in_=agg_ps[:])

                h1a_ps = ps_a.tile([P, P], _F32, tag="h1a")
                h1b_ps = ps_b.tile([P, P], _F32, tag="h1b")
                nc.tensor.matmul(
                    out=h1a_ps[:], lhsT=w1_sb[:, :P], rhs=agg[:], start=True, stop=False
                )
                nc.tensor.matmul(
                    out=h1a_ps[:],
                    lhsT=b1_sb[:1, :P],
                    rhs=ones_sb[:1, :],
                    start=False,
                    stop=True,
                )
                nc.tensor.matmul(
                    out=h1b_ps[:], lhsT=w1_sb[:, P:], rhs=agg[:], start=True, stop=False
                )
                nc.tensor.matmul(
                    out=h1b_ps[:],
                    lhsT=b1_sb[:1, P:],
                    rhs=ones_sb[:1, :],
                    start=False,
                    stop=True,
                )
                r1a = wp.tile([P, P], _BF16, tag="r1a")
                r1b = wp.tile([P, P], _BF16, tag="r1b")
                nc.vector.tensor_scalar_max(out=r1a[:], in0=h1a_ps[:], scalar1=0.0)
                nc.vector.tensor_scalar_max(out=r1b[:], in0=h1b_ps[:], scalar1=0.0)

                h2_ps = ps_h2.tile([P, D_OUT], _F32, tag="h2")
                nc.tensor.matmul(
                    out=h2_ps[:], lhsT=r1a[:], rhs=w2a_sb[:], start=True, stop=False
                )
                nc.tensor.matmul(
                    out=h2_ps[:], lhsT=r1b[:], rhs=w2b_sb[:], start=False, stop=True
                )
                h2w = wp.tile([P, D_OUT], _BF16, tag="h2w")
                nc.vector.tensor_copy(out=h2w[:], in_=h2_ps[:])
                h2_writes[w] = nc.sync.dma_start(
                    out=h2s[w * P : (w + 1) * P, :], in_=h2w[:]
                )

            cc = nc.gpsimd.collective_compute(
                "AllGather",
                mybir.AluOpType.bypass,
                ins=[h2s[:]],
                outs=[h2f[:]],
                replica_groups=[list(range(NCORES))],
            )

            # ---- layer 2: out = A_hat @ h2_full + b2 ----
            for w in range(WINDOWS):
                Kw = int(Ks[w])
                o_ps = ps_acc.tile([P, D_OUT], _F32, tag="acc")
                h2loc = wp.tile([P, D_OUT], _BF16, tag="h2loc")
                rb = nc.sync.dma_start(
                    out=h2loc[:], in_=h2s[w * P : (w + 1) * P, :]
                )
                add_dep_helper(
                    rb.ins, h2_writes[w].ins, reason="read back own h2 piece"
                )
                dg = build_diag(w)
                nc.tensor.matmul(
                    out=o_ps[:],
                    lhsT=ones_sb[:1, :],
                    rhs=b2_sb[:1, :],
                    start=True,
                    stop=False,
                )
                nc.tensor.matmul(
                    out=o_ps[:], lhsT=dg[:], rhs=h2loc[:], start=False, stop=False
                )
                for k in range(Kw):
                    c = int(bases[w]) + k
                    m2 = wp.tile([P, D_OUT], _BF16, tag="m2")
                    g = nc.gpsimd.indirect_dma_start(
                        out=m2[:],
                        out_offset=None,
                        in_=h2f[:],
                        in_offset=bass.IndirectOffsetOnAxis(
                            ap=idx2_sb[:, c : c + 1], axis=0
                        ),
                    )
                    add_dep_helper(g.ins, cc.ins, reason="gather reads AllGather out")
                    s = wp.tile([P, P], _BF16, tag="s1")
                    nc.vector.tensor_scalar(
                        out=s[:],
                        in0=iota_sb[:],
                        scalar1=dstw_sb[:, c : c + 1],
                        scalar2=nrm_sb[:, c : c + 1],
                        op0=mybir.AluOpType.is_equal,
                        op1=mybir.AluOpType.mult,
                    )
                    nc.tensor.matmul(
                        out=o_ps[:],
                        lhsT=s[:],
                        rhs=m2[:],
                        start=False,
                        stop=(k == Kw - 1),
                    )
                o = wp.tile([P, D_OUT], _F32, tag="o")
                nc.vector.tensor_copy(out=o[:], in_=o_ps[:])
                nc.sync.dma_start(out=out[w * P : (w + 1) * P, :], in_=o[:])

    _split_multi_waits(nc)
    return nc


def kernel(x, edge_index, W1, b1, W2, b2):
    global LAST_EXEC_NS, LAST_RESULTS
    x = np.ascontiguousarray(np.asarray(x, dtype=np.float32).astype(bfloat16))
    W1 = np.ascontiguousarray(np.asarray(W1, dtype=np.float32).astype(bfloat16))
    W2 = np.asarray(W2, dtype=np.float32).astype(bfloat16)
    b1 = np.asarray(b1, dtype=np.float32).astype(bfloat16).reshape(1, HID)
    b2 = np.asarray(b2, dtype=np.float32).astype(bfloat16).reshape(1, D_OUT)

    Ks, bases, perm, dinv, idx1, idx2, dstw, nrm = _build_schedule(
        np.asarray(edge_index)
    )
    nc = _build_bass(Ks, bases)

    iota = np.tile(np.arange(P, dtype=np.float32), (P, 1))
    iotac = np.arange(P, dtype=np.float32).reshape(P, 1)
    w2a = np.ascontiguousarray(W2[:P])
    w2b = np.ascontiguousarray(W2[P:])

    # per-core slot-ordered x rows + squared dinv (0 on padded tail rows)
    xlocs, dinv2s = [], []
    for c in range(NCORES):
        nodes = (
            c * D_CORE
            + perm[c][:, None] * P
            + np.arange(P)[None, :]
        ).reshape(-1)
        valid = nodes < (c + 1) * D_CORE
        nv = np.where(valid, nodes, 0)
        xl = np.where(valid[:, None], x[nv].astype(np.float32), 0.0).astype(
            bfloat16
        )
        d2 = np.where(valid, dinv[nv] ** 2, 0.0).astype(np.float32)
        xlocs.append(np.ascontiguousarray(xl))
        # dinv2 layout [P, WINDOWS]: partition r, slot w
        dinv2s.append(np.ascontiguousarray(d2.reshape(WINDOWS, P).T))

    in_maps = []
    for c in range(NCORES):
        in_maps.append(
            {
                "x": x,
                "w1": W1,
                "w2a": w2a,
                "w2b": w2b,
                "b1": b1,
                "b2": b2,
                "iota": iota,
                "xloc": xlocs[c],
                "dinv2": dinv2s[c],
                "iotac": iotac,
                "idx1": np.ascontiguousarray(idx1[c]),
                "idx2": np.ascontiguousarray(idx2[c]),
                "dstw": np.ascontiguousarray(dstw[c]),
                "nrm": np.ascontiguousarray(nrm[c]),
            }
        )

    res = bass_utils.run_bass_kernel_spmd(
        nc, in_maps, core_ids=list(range(NCORES)), trace=TRACE
    )
    LAST_EXEC_NS = res.exec_time_ns
    LAST_RESULTS = res

    # un-permute: out row slot*P + d -> node perm[c][slot]*P + d
    shards = []
    for c in range(NCORES):
        a = res.results[c]["out"].reshape(WINDOWS, P, D_OUT)
        inv = np.empty(WINDOWS, np.int64)
        inv[perm[c]] = np.arange(WINDOWS)
        shards.append(a[inv].reshape(D_PAD, D_OUT)[:D_CORE])
    return np.concatenate(shards, axis=0)
